# revision 1
# baseline (speedup 1.0000x reference)
"""Trainium2 Bass kernel for nn_BlastLinear (block low-rank linear layer).

Math (reference):
  y[q,n,r] = sum_c x[n, q*1024+c] * C[q,r,c]          (mm1, per input block q)
  z[p,n,r] = sum_q D[p,q,r] * y[q,n,r]                (tiny mix over q)
  o[p,n,j] = sum_r z[p,n,r] * B[p,j,r]                (mm2, per output block p)
  out[n, p*1024+j] = o[p,n,j] + bias[p*1024+j]

Sharding: pure data-parallel over the 8192 tokens -> 1024 tokens per core,
weights replicated, no collectives.

Precision: the PE's fast fp32 path (float32r) keeps only 12 significand
bits, so every operand A is split on the host (or on-chip for z) into
A = Ah + Al with both parts exactly f32r-representable, and each matmul
runs three f32r passes accumulating in the same PSUM group:
  A@X ~= Ah@Xh + Ah@Xl + Al@Xh      (drops only Al@Xl ~ 2^-24)
This is 3 cycles/row vs 4 for the native fp32 path, with ~1e-7 rel error.

Per-core pipeline (chunk = 512 tokens, 2 chunks):
  mm1:  psum y^T[q,rt] [128r x 512n] += 3-pass over k of ct^T @ xt  (PE)
  mix:  z[p,rt] = sum_q D[p,q,rt-slice] * y^T[q,rt]   (DVE fused mul-add,
        reads y straight from PSUM, accumulates fp32 in SBUF)
  split: zh = f32r(z), zl = z - zh                    (DVE)
  mm2:  psum o[mt,oc] = 3-pass over rt of z^T @ bt    (PE)
  out:  DVE drains psum -> SBUF fused with the bias add
        (bias pre-broadcast once into resident [128,512] tiles), DMA out.
ct_hi stays resident in SBUF; ct_lo / bt_hi / bt_lo stream per chunk.
TimelineSim (the CoreSim cost model): ~366 us/core, PE-bound at 92% with
PE busy at the 3-pass floor; modeled DMA ~295 us (~96 MiB; ct_lo ships as bf16 for the third mm1 pass, with a bf16 twin of x_hi cast on GPSIMD).
"""

import numpy as np

import concourse.mybir as mybir
import concourse.tile as tile
from concourse import bacc
from concourse.bass_utils import run_bass_kernel_spmd

N_CORES = 8
IN_F = 4096
OUT_F = 4096
P = 4
Q = 4
R = 512
CB = IN_F // Q        # 1024 input features per q block
OB = OUT_F // P       # 1024 output features per p block
N_TOK = 4 * 2048      # 8192 total tokens
N_CORE = N_TOK // N_CORES   # 1024 tokens per core

CHUNK = 512           # tokens per pipeline chunk
KT1 = CB // 128       # 8 contraction tiles per q in mm1
RT = R // 128         # 4 rank partition tiles
KB = 2                # k-tiles per x DMA batch

F32 = mybir.dt.float32
F32R = mybir.dt.float32r
BF16 = mybir.dt.bfloat16
MULT = mybir.AluOpType.mult
ADD = mybir.AluOpType.add
SUB = mybir.AluOpType.subtract

_cached_nc = None


def round_f32r(a):
    """Round fp32 array to f32r (12-bit significand), RTNE at bit 12."""
    u = np.ascontiguousarray(a, dtype=np.float32).view(np.uint32)
    lsb = (u >> 12) & np.uint32(1)
    u2 = (u + np.uint32(0x7FF) + lsb) & np.uint32(0xFFFFF000)
    return u2.view(np.float32)


def _build(n_core=N_CORE, chunk=CHUNK):
    nc = bacc.Bacc("TRN2", target_bir_lowering=False, debug=False,
                   enable_asserts=False)

    def din(name, shape, dtype=F32R):
        return nc.dram_tensor(name, shape, dtype, kind="ExternalInput").ap()

    xth = din("xth", [IN_F, n_core])
    xtl = din("xtl", [IN_F, n_core])
    cth = din("cth", [IN_F, R])
    ctl = din("ctl", [IN_F, R], BF16)
    bth = din("bth", [P * R, OB])
    btl = din("btl", [P * R, OB])
    dr = din("dr", [R, P * Q], F32)
    bias2 = din("bias2", [2, OUT_F])   # row 0: hi, row 1: lo
    onesd = din("onesd", [2, 128])
    out = nc.dram_tensor("out", [n_core, OUT_F], F32,
                         kind="ExternalOutput").ap()

    n_chunks = n_core // chunk
    MT = chunk // 128     # mm2 token tiles per chunk
    OC = OB // 512        # output free-dim chunks per p

    with tile.TileContext(nc) as tc:
        with (
            tc.tile_pool(name="const", bufs=1) as cpool,
            tc.tile_pool(name="ctlp", bufs=2) as ctlpool,
            tc.tile_pool(name="xp", bufs=3) as xpool,
            tc.tile_pool(name="btp", bufs=6) as btpool,
            tc.tile_pool(name="zp", bufs=16) as zpool,
            tc.tile_pool(name="zhp", bufs=7) as zhpool,
            tc.tile_pool(name="outp", bufs=3) as outpool,
            tc.tile_pool(name="biasp", bufs=1) as biaspool,
            tc.tile_pool(name="yps", bufs=6, space="PSUM") as ypool,
            tc.tile_pool(name="ops", bufs=2, space="PSUM") as opool,
        ):
            # cth_sb[p, q*8+k, r]: C^T_hi tile rows c = (q*8+k)*128 + p
            # DMA'd piecewise inside chunk 0's loop so matmuls start early.
            cth_sb = cpool.tile([128, IN_F // 128, R], F32R)
            cth3 = cth.rearrange("(t p) r -> p t r", p=128)
            # d_sb[p_, rt, p*4 + q] = D[p, q, rt*128 + p_]
            # (DMA'd after the first x tiles; see emit_mm1 j=0/q=0)
            d_sb = cpool.tile([128, RT, P * Q], F32)
            ones_sb = cpool.tile([2, 128], F32R)

            z = {}
            zsplit = {}
            bias_bc = {}

            def emit_bias_bc():
                # broadcast bias (hi+lo, exact) to [128, 512] tiles once;
                # mm2 then folds the add into the DVE psum drain
                for p in range(P):
                    for oc in range(OC):
                        off = p * OB + oc * 512
                        b2_t = biaspool.tile([2, 512], F32R, tag="bi2",
                                             name=f"bi2_{p}_{oc}")
                        nc.sync.dma_start(b2_t[:], bias2[0:2, off:off + 512])
                        bps = opool.tile([128, 512], F32, tag="o",
                                         name=f"bps_{p}_{oc}")
                        nc.tensor.matmul(ops := bps[:], lhsT=ones_sb[:],
                                         rhs=b2_t[:], start=True, stop=True)
                        bc = cpool.tile([128, 512], F32, tag=f"bc_{p}_{oc}",
                                        name=f"bc_{p}_{oc}")
                        nc.scalar.copy(bc[:], ops)
                        bias_bc[(p, oc)] = bc

            def emit_zsplit(j, p, rt):
                # cast on ACT (idle) keeps the DVE critical chain short;
                # the subtract stays on DVE.
                zt = z.pop((j, p, rt))
                zh_t = zhpool.tile([128, chunk], F32R, tag="zh",
                                   name=f"zh_{j}_{p}_{rt}")
                nc.scalar.copy(zh_t[:], zt[:])
                zl_t = zhpool.tile([128, chunk], F32R, tag="zl",
                                   name=f"zl_{j}_{p}_{rt}")
                nc.gpsimd.tensor_tensor(
                    zl_t[:], zt[:], zh_t[:].bitcast(F32), op=SUB)
                zsplit[(j, p, rt)] = (zh_t, zl_t)

            def emit_bt_dma(j, p, oc):
                off = p * OB + oc * 512
                hts, lts = [], []
                for rt in range(RT):
                    rb = p * R + rt * 128
                    bth_t = btpool.tile([128, 512], F32R, tag="bth",
                                        name=f"bth_{j}_{p}_{oc}_{rt}")
                    nc.sync.dma_start(
                        bth_t[:], bth[rb:rb + 128, oc * 512:(oc + 1) * 512])
                    hts.append(bth_t)
                    btl_t = btpool.tile([128, 512], F32R, tag="btl",
                                        name=f"btl_{j}_{p}_{oc}_{rt}")
                    nc.sync.dma_start(
                        btl_t[:], btl[rb:rb + 128, oc * 512:(oc + 1) * 512])
                    lts.append(btl_t)
                return hts, lts

            bt_pre = {}

            def emit_mm1(j):
                for q in range(Q):
                    if j == 0 and q > 0:
                        qs = slice(q * KT1, (q + 1) * KT1)
                        nc.sync.dma_start(cth_sb[:, qs, :], cth3[:, qs, :])
                    ys = [
                        ypool.tile([128, chunk], F32, tag="y",
                                   name=f"y_{j}_{q}_{rt}")
                        for rt in range(RT)
                    ]
                    for kb in range(KT1 // KB):
                        if j == 0 and q == 0:
                            # q0's cth piece rides just ahead of its own
                            # kb's x tiles, so the first matmul waits on
                            # ~1 MiB of DMA, not the whole 2 MiB of q0
                            hs = slice(kb * KB, (kb + 1) * KB)
                            nc.sync.dma_start(cth_sb[:, hs, :],
                                              cth3[:, hs, :])
                        if j == 0 and q == 0 and kb == 1:
                            nc.sync.dma_start(
                                d_sb[:],
                                dr.rearrange("(t p) s -> p t s", p=128))
                            nc.sync.dma_start(ones_sb[:], onesd[:])
                        if j == 0 and q == 1 and kb == 0:
                            emit_bias_bc()
                        if q == Q - 1 and kb == 2:
                            # prefetch first mm2 weight group late in q3,
                            # after q3's own x DMAs are underway
                            bt_pre[(j, 0, 0)] = emit_bt_dma(j, 0, 0)
                        base = (q * KT1 + kb * KB) * 128
                        xh_t = xpool.tile([128, KB, chunk], F32R, tag="xh",
                                          name=f"xh_{j}_{q}_{kb}")
                        xl_t = xpool.tile([128, KB, chunk], F32R, tag="xl",
                                          name=f"xl_{j}_{q}_{kb}")
                        first = j == 0 and q == 0 and kb == 0
                        for src_d, t in ((xth, xh_t), (xtl, xl_t)):
                            # per-k pieces at kernel start so the first
                            # matmul waits on ~512 KiB, not the full batch
                            pieces = KB if first else 1
                            for pc in range(pieces):
                                w = KB // pieces
                                nc.sync.dma_start(
                                    t[:, pc * w:(pc + 1) * w, :],
                                    src_d[base + pc * w * 128:
                                          base + (pc + 1) * w * 128,
                                          j * chunk:(j + 1) * chunk]
                                    .rearrange("(t p) n -> p t n", p=128))
                        ctl_t = ctlpool.tile([128, KB, R], BF16, tag="ctl",
                                             name=f"ctl_{j}_{q}_{kb}")
                        nc.sync.dma_start(
                            ctl_t[:],
                            ctl[base:base + KB * 128, :]
                            .rearrange("(t p) r -> p t r", p=128))
                        # bf16 twin of xh for the bf16 lo-weight pass
                        xhb_t = xpool.tile([128, KB, chunk], BF16, tag="xhb",
                                           name=f"xhb_{j}_{q}_{kb}", bufs=2)
                        nc.gpsimd.tensor_copy(
                            xhb_t[:], xh_t[:].bitcast(F32))
                        for rt in range(RT):
                            for kk in range(KB):
                                k = kb * KB + kk
                                hi_w = cth_sb[:, q * KT1 + k,
                                              rt * 128:(rt + 1) * 128]
                                lo_w = ctl_t[:, kk, rt * 128:(rt + 1) * 128]
                                nc.tensor.matmul(
                                    ys[rt][:], lhsT=hi_w, rhs=xh_t[:, kk, :],
                                    start=(k == 0), stop=False)
                                nc.tensor.matmul(
                                    ys[rt][:], lhsT=hi_w, rhs=xl_t[:, kk, :],
                                    start=False, stop=False)
                                nc.tensor.matmul(
                                    ys[rt][:], lhsT=lo_w, rhs=xhb_t[:, kk, :],
                                    start=False, stop=(k == KT1 - 1))
                    # rt-major frees each y PSUM bank after 4 ops; on the
                    # last q, split z into f32r hi/lo right after its final
                    # accumulation so mm2 isn't gated on a DVE tail.
                    for rt in range(RT):
                        for p in range(P):
                            col = p * Q + q
                            dcol = d_sb[:, rt, col:col + 1]
                            if q == 0:
                                zt = zpool.tile([128, chunk], F32, tag="z",
                                                name=f"z_{j}_{p}_{rt}")
                                z[(j, p, rt)] = zt
                                nc.vector.tensor_scalar_mul(
                                    zt[:], ys[rt][:], dcol)
                            else:
                                zt = z[(j, p, rt)]
                                nc.vector.scalar_tensor_tensor(
                                    zt[:], ys[rt][:], dcol, zt[:],
                                    op0=MULT, op1=ADD)
                            if q == Q - 1 and p == 0:
                                # eager split for p0 only: it gates mm2 start
                                emit_zsplit(j, p, rt)

            def emit_mm2(j):
                for p in range(P):
                    for rt in range(RT):
                        if (j, p, rt) not in zsplit:
                            emit_zsplit(j, p, rt)
                    zh = {rt: zsplit[(j, p, rt)][0] for rt in range(RT)}
                    zl = {rt: zsplit[(j, p, rt)][1] for rt in range(RT)}
                    for oc in range(OC):
                        off = p * OB + oc * 512
                        if (j, p, oc) in bt_pre:
                            bth_ts, btl_ts = bt_pre.pop((j, p, oc))
                        else:
                            bth_ts, btl_ts = emit_bt_dma(j, p, oc)
                        for mt in range(MT):
                            ops = opool.tile([128, 512], F32, tag="o",
                                             name=f"o_{j}_{p}_{oc}_{mt}")
                            ms = slice(mt * 128, (mt + 1) * 128)
                            for rt in range(RT):
                                nc.tensor.matmul(
                                    ops[:], lhsT=zh[rt][:, ms],
                                    rhs=bth_ts[rt][:],
                                    start=(rt == 0), stop=False)
                                nc.tensor.matmul(
                                    ops[:], lhsT=zh[rt][:, ms],
                                    rhs=btl_ts[rt][:],
                                    start=False, stop=False)
                                nc.tensor.matmul(
                                    ops[:], lhsT=zl[rt][:, ms],
                                    rhs=bth_ts[rt][:],
                                    start=False, stop=(rt == RT - 1))
                            ot = outpool.tile([128, 512], F32, tag="ot",
                                              name=f"ot_{j}_{p}_{oc}_{mt}")
                            nc.vector.tensor_tensor(
                                ot[:], ops[:], bias_bc[(p, oc)][:], op=ADD)
                            nc.sync.dma_start(
                                out[j * chunk + mt * 128:
                                    j * chunk + (mt + 1) * 128,
                                    off:off + 512],
                                ot[:])

            for j in range(n_chunks):
                emit_mm1(j)
                emit_mm2(j)

    nc.compile()
    return nc


def _prep_in_maps(x, B, C, D, bias):
    x2 = np.ascontiguousarray(
        np.asarray(x, dtype=np.float32).reshape(N_TOK, IN_F))
    CT = np.ascontiguousarray(
        np.asarray(C, dtype=np.float32).transpose(0, 2, 1).reshape(IN_F, R))
    BT = np.ascontiguousarray(
        np.asarray(B, dtype=np.float32).transpose(0, 2, 1).reshape(P * R, OB))
    DR = np.ascontiguousarray(
        np.asarray(D, dtype=np.float32).transpose(2, 0, 1).reshape(R, P * Q))
    bias2 = np.ascontiguousarray(
        np.asarray(bias, dtype=np.float32).reshape(1, OUT_F))

    import ml_dtypes
    CTH = round_f32r(CT)
    CTL = np.ascontiguousarray((CT - CTH).astype(ml_dtypes.bfloat16))
    BTH = round_f32r(BT)
    BTL = np.ascontiguousarray(BT - BTH)
    BIH = round_f32r(bias2)
    BI2 = np.ascontiguousarray(
        np.concatenate([BIH, bias2 - BIH], axis=0))
    ONES = np.ones((2, 128), dtype=np.float32)

    in_maps = []
    for c in range(N_CORES):
        xt = np.ascontiguousarray(x2[c * N_CORE:(c + 1) * N_CORE].T)
        xh = round_f32r(xt)
        xl = np.ascontiguousarray(xt - xh)
        in_maps.append({
            "xth": xh, "xtl": xl, "cth": CTH, "ctl": CTL,
            "bth": BTH, "btl": BTL, "dr": DR,
            "bias2": BI2, "onesd": ONES,
        })
    return in_maps


def _run(in_maps, trace=False):
    global _cached_nc
    if _cached_nc is None:
        _cached_nc = _build()
    import time
    for attempt in range(3):
        try:
            return run_bass_kernel_spmd(
                _cached_nc, in_maps, list(range(N_CORES)), trace=trace)
        except Exception:
            # transient device errors (e.g. NRT_EXEC_UNIT_UNRECOVERABLE
            # from a previously wedged core) usually clear on retry
            if attempt == 2:
                raise
            time.sleep(5.0 * (attempt + 1))


def kernel(x, B, C, D, bias):
    lead = np.asarray(x).shape[:-1]
    res = _run(_prep_in_maps(x, B, C, D, bias))
    outs = [res.results[c]["out"] for c in range(N_CORES)]
    return np.concatenate(outs, axis=0).reshape(*lead, OUT_F)



# revision 11
# speedup vs baseline: 2.8837x; 2.8837x over previous
"""Trainium2 Bass kernel for nn_BlastLinear (block low-rank linear layer).

Math (reference):
  y[q,n,r] = sum_c x[n, q*1024+c] * C[q,r,c]          (mm1, per input block q)
  z[p,n,r] = sum_q D[p,q,r] * y[q,n,r]                (tiny mix over q)
  o[p,n,j] = sum_r z[p,n,r] * B[p,j,r]                (mm2, per output block p)
  out[n, p*1024+j] = o[p,n,j] + bias[p*1024+j]

Sharding: pure data-parallel over the 8192 tokens -> 1024 tokens per core,
weights replicated, no collectives.

Precision: single-pass bf16 matmuls with fp32 PSUM accumulation. The
tolerance for this problem is 2e-2; bf16 inputs give ~2.4e-3 (measured on
the real input distribution), so the 3-pass f32r split scheme is wasted
work. This puts the PE at its 1-cycle/row floor: 8.59 GFLOP/core at
78.6 TF/s ~= 109 us.

Layout: everything keeps [feature-or-rank on partitions, tokens moving]:
  mm1:  psum y[rt] [128r x 512n] += ct[kt,rt] @ xt[kt]        (PE, bf16)
  mix:  ACT drains each y PSUM bank to SBUF once (frees the bank after a
        single read instead of four), then z[p,rt] (bf16, SBUF) is
        accumulated in-place over q on DVE; with all operands in SBUF the
        DVE runs its 2x perf mode.
  mm2:  psum o[fb] [128o x 512n] += bt[p,rt,fb] @ z[p,rt]     (PE, bf16)
  out:  ACT drains psum -> SBUF fused with the bias add (bias is
        per-partition in this orientation), DMA out feature-major;
        the host transposes.

Chunks of 512 tokens; emission order mm1(0), mm1(1), mm2(0), mm2(1) so the
mix tail of each chunk hides under the other chunk's matmuls. DMA order:
x(0)+C interleaved, then x(1) interleaved with B, then out. ~32 MiB total
HBM traffic (~93 us) stays under the PE time.
"""

import numpy as np

import concourse.mybir as mybir
import concourse.tile as tile
from concourse import bacc
from concourse.bass_utils import run_bass_kernel_spmd

N_CORES = 8
IN_F = 4096
OUT_F = 4096
P = 4
Q = 4
R = 512
CB = IN_F // Q        # 1024 input features per q block
OB = OUT_F // P       # 1024 output features per p block
N_TOK = 4 * 2048      # 8192 total tokens
N_CORE = N_TOK // N_CORES   # 1024 tokens per core

CHUNK = 512           # tokens per pipeline chunk
KT1 = CB // 128       # 8 contraction tiles per q in mm1
RT = R // 128         # 4 rank partition tiles
FB = OB // 128        # 8 output-feature tiles per p in mm2

F32 = mybir.dt.float32
BF16 = mybir.dt.bfloat16
MULT = mybir.AluOpType.mult
ADD = mybir.AluOpType.add
IDENT = mybir.ActivationFunctionType.Identity

_cached_nc = None


def _build(n_core=N_CORE, chunk=CHUNK):
    nc = bacc.Bacc("TRN2", target_bir_lowering=False, debug=False,
                   enable_asserts=False)

    def din(name, shape, dtype):
        return nc.dram_tensor(name, shape, dtype, kind="ExternalInput").ap()

    xtb = din("xtb", [IN_F, n_core], BF16)
    ctb = din("ctb", [IN_F, R], BF16)
    btb = din("btb", [P * R, OB], BF16)
    dr = din("dr", [R, P * Q], F32)
    biasr = din("biasr", [128, OUT_F // 128], F32)
    out = nc.dram_tensor("out", [OUT_F, n_core], BF16,
                         kind="ExternalOutput").ap()

    n_chunks = n_core // chunk
    jB = min(1, n_chunks - 1)   # chunk whose mm1 streams in the B tiles

    with tile.TileContext(nc) as tc:
        with (
            tc.tile_pool(name="const", bufs=1) as cpool,
            tc.tile_pool(name="xp", bufs=2) as xpool,
            tc.tile_pool(name="zp", bufs=16 * min(2, n_chunks)) as zpool,
            tc.tile_pool(name="outp", bufs=6) as outpool,
            tc.tile_pool(name="ybp", bufs=6) as ybpool,
            tc.tile_pool(name="yps", bufs=6, space="PSUM") as ypool,
            tc.tile_pool(name="ops", bufs=2, space="PSUM") as opool,
        ):
            # ct_sb[p, t, r]: C^T rows c = t*128 + p (t = q*KT1 + kt)
            ct_sb = cpool.tile([128, IN_F // 128, R], BF16)
            ct3 = ctb.rearrange("(t p) r -> p t r", p=128)
            # bt_sb[p, t, o]: B^T rows (t = p_blk*RT + rt)
            bt_sb = cpool.tile([128, (P * R) // 128, OB], BF16)
            bt3 = btb.rearrange("(t p) o -> p t o", p=128)
            # d_sb[p_, rt, p*Q + q] = D[p, q, rt*128 + p_]
            d_sb = cpool.tile([128, RT, P * Q], F32)
            bias_sb = cpool.tile([128, OUT_F // 128], F32)

            z = {}
            x_sb = {}

            def emit_mm1(j):
                x_sb[j] = xpool.tile([128, IN_F // 128, chunk], BF16,
                                     tag="x", name=f"x_{j}")
                xt = x_sb[j]
                for q in range(Q):
                    ys = [
                        ypool.tile([128, chunk], F32, tag="y",
                                   name=f"y_{j}_{q}_{rt}")
                        for rt in range(RT)
                    ]
                    # x (and, on the right chunks, C / B) stream in pieces
                    # just ahead of the matmuls that consume them.
                    first = j == 0 and q == 0
                    # piece boundaries (kt, width): a tiny first piece on
                    # the very first q so the PE starts ~1.5us earlier
                    pieces = dict(((0, 1), (1, 3), (4, 4))) if first else \
                        dict(((0, 4), (4, 4)))
                    for kt in range(KT1):
                        if kt in pieces:
                            w = pieces[kt]
                            t0 = q * KT1 + kt
                            ts = slice(t0, t0 + w)
                            rows = slice(t0 * 128, (t0 + w) * 128)
                            nc.sync.dma_start(
                                xt[:, ts, :],
                                xtb[rows, j * chunk:(j + 1) * chunk]
                                .rearrange("(t p) n -> p t n", p=128))
                            if j == 0:
                                nc.sync.dma_start(ct_sb[:, ts, :],
                                                  ct3[:, ts, :])
                        if first and kt == 1:
                            # tiny constants ride behind the first pieces
                            nc.sync.dma_start(
                                d_sb[:],
                                dr.rearrange("(t p) s -> p t s", p=128))
                            nc.sync.dma_start(bias_sb[:], biasr[:, :])
                        if j == jB and kt == KT1 // 2:
                            # one quarter of B rides behind each q's x
                            bs = slice(q * RT, (q + 1) * RT)
                            nc.sync.dma_start(bt_sb[:, bs, :], bt3[:, bs, :])
                        for rt in range(RT):
                            nc.tensor.matmul(
                                ys[rt][:],
                                lhsT=ct_sb[:, q * KT1 + kt,
                                           rt * 128:(rt + 1) * 128],
                                rhs=xt[:, q * KT1 + kt, :],
                                start=(kt == 0), stop=(kt == KT1 - 1))
                    # mix: z[p,rt] += D[p,q,rt-slice] * y[rt], in-place bf16.
                    # ACT drains each y PSUM bank to SBUF in one read (so
                    # the bank frees fast), then the 4 per-p multiplies run
                    # on DVE from SBUF where its 2x perf mode applies.
                    for rt in range(RT):
                        yb = ybpool.tile([128, chunk], F32, tag="yb",
                                         name=f"yb_{j}_{q}_{rt}")
                        nc.scalar.copy(yb[:], ys[rt][:])
                        for p in range(P):
                            if q == 0:
                                zt = zpool.tile([128, chunk], BF16, tag="z",
                                                name=f"z_{j}_{p}_{rt}")
                                z[(j, p, rt)] = zt
                                nc.vector.tensor_scalar_mul(
                                    zt[:], yb[:],
                                    d_sb[:, rt, p * Q:p * Q + 1])
                            else:
                                zt = z[(j, p, rt)]
                                nc.vector.scalar_tensor_tensor(
                                    zt[:], yb[:],
                                    d_sb[:, rt, p * Q + q:p * Q + q + 1],
                                    zt[:], op0=MULT, op1=ADD)

            def emit_mm2(j):
                for p in range(P):
                    for fb in range(FB):
                        ops = opool.tile([128, chunk], F32, tag="o",
                                         name=f"o_{j}_{p}_{fb}")
                        for rt in range(RT):
                            nc.tensor.matmul(
                                ops[:],
                                lhsT=bt_sb[:, p * RT + rt,
                                           fb * 128:(fb + 1) * 128],
                                rhs=z[(j, p, rt)][:],
                                start=(rt == 0), stop=(rt == RT - 1))
                        ot = outpool.tile([128, chunk], BF16, tag="ot",
                                          name=f"ot_{j}_{p}_{fb}")
                        t = p * FB + fb
                        nc.scalar.activation(ot[:], ops[:], IDENT,
                                             bias=bias_sb[:, t:t + 1],
                                             scale=1.0)
                        nc.sync.dma_start(
                            out[t * 128:(t + 1) * 128,
                                j * chunk:(j + 1) * chunk],
                            ot[:])
                for p in range(P):
                    for rt in range(RT):
                        del z[(j, p, rt)]

            # mm1 chunks back-to-back, then the mm2s: each chunk's DVE mix
            # tail hides under the other chunk's matmuls.
            for j in range(n_chunks):
                emit_mm1(j)
            for j in range(n_chunks):
                emit_mm2(j)

    nc.compile()
    return nc


def _prep_in_maps(x, B, C, D, bias):
    import ml_dtypes
    BF = ml_dtypes.bfloat16
    x2 = np.asarray(x, dtype=np.float32).reshape(N_TOK, IN_F)
    CTB = np.ascontiguousarray(
        np.asarray(C, dtype=np.float32).transpose(0, 2, 1)
        .reshape(IN_F, R).astype(BF))
    BTB = np.ascontiguousarray(
        np.asarray(B, dtype=np.float32).transpose(0, 2, 1)
        .reshape(P * R, OB).astype(BF))
    DR = np.ascontiguousarray(
        np.asarray(D, dtype=np.float32).transpose(2, 0, 1).reshape(R, P * Q))
    BIASR = np.ascontiguousarray(
        np.asarray(bias, dtype=np.float32).reshape(OUT_F // 128, 128).T)

    in_maps = []
    for c in range(N_CORES):
        xt = np.ascontiguousarray(
            x2[c * N_CORE:(c + 1) * N_CORE].T.astype(BF))
        in_maps.append({
            "xtb": xt, "ctb": CTB, "btb": BTB, "dr": DR, "biasr": BIASR,
        })
    return in_maps


def _run(in_maps, trace=False):
    global _cached_nc
    if _cached_nc is None:
        _cached_nc = _build()
    import time
    for attempt in range(3):
        try:
            return run_bass_kernel_spmd(
                _cached_nc, in_maps, list(range(N_CORES)), trace=trace)
        except Exception:
            # transient device errors (e.g. NRT_EXEC_UNIT_UNRECOVERABLE
            # from a previously wedged core) usually clear on retry
            if attempt == 2:
                raise
            time.sleep(5.0 * (attempt + 1))


def kernel(x, B, C, D, bias):
    lead = np.asarray(x).shape[:-1]
    res = _run(_prep_in_maps(x, B, C, D, bias))
    # per-core outputs are feature-major bf16 [OUT_F, n_core]; transpose back
    outs = [np.ascontiguousarray(res.results[c]["out"].T.astype(np.float32))
            for c in range(N_CORES)]
    return np.concatenate(outs, axis=0).reshape(*lead, OUT_F)


# revision 28
# speedup vs baseline: 3.1200x; 1.0819x over previous
"""Trainium2 Bass kernel for nn_BlastLinear (block low-rank linear layer).

Math (reference):
  y[q,n,r] = sum_c x[n, q*1024+c] * C[q,r,c]          (mm1, per input block q)
  z[p,n,r] = sum_q D[p,q,r] * y[q,n,r]                (tiny mix over q)
  o[p,n,j] = sum_r z[p,n,r] * B[p,j,r]                (mm2, per output block p)
  out[n, p*1024+j] = o[p,n,j] + bias[p*1024+j]

Sharding: pure data-parallel over the 8192 tokens -> 1024 tokens per core,
weights replicated, no collectives.

Precision: fp8(e4m3) DoubleRow matmuls, 3-term hi/lo per GEMM:
  A@X ~= Ah@Xh + Ah@Xl + Al@Xh     (drops Al@Xl ~ (2^-4)^2)
DoubleRow packs K=256 per instruction at 0.5 cycles/row, so the 3-term
scheme costs 0.75x a single bf16 pass while keeping ~0.3% per-stage rms
error (measured end-to-end ~3.3e-3 against the 2e-2 tolerance). e4m3
subnormals would wreck the small-magnitude weights, so the host pre-scales
x by 8 and C/B by 64 into the e4m3 normal range and the inverse scales
fold into the D mix constants (D/8) and the output drain (scale 1/4096).

Layout keeps [feature-or-rank on partitions, tokens moving] end to end:
  mm1:  psum y[rt] [128r x 512n] += 3-term DoubleRow over K-pairs  (PE)
  mix:  ACT drains each y bank to SBUF f32 once (fast PSUM-bank free),
        z'[p,rt] (bf16) accumulated in-place over q on DVE
  split: zh8 = e4m3(z'), zl8 = e4m3(z' - zh8) on GPSIMD (Pool), written
        into [128, 2, 512] pair tiles shaped for DoubleRow rhs
  mm2:  psum o[fb] [128o x 512n] += 3-term DoubleRow over rank pairs (PE)
  out:  ACT drain fused with 1/4096 un-scale + per-partition bias add,
        bf16 out DMA'd feature-major; the host transposes/upcasts.

Chunks of 512 tokens; emission order mm1(0), mm1(1), mm2(0), mm2(1) so
each chunk's mix/split tail hides under the other chunk's matmuls.
~24 MiB HBM traffic (~70 us) under ~86 us of PE time.
"""

import numpy as np

import concourse.mybir as mybir
import concourse.tile as tile
from concourse import bacc
from concourse.bass_utils import run_bass_kernel_spmd

N_CORES = 8
IN_F = 4096
OUT_F = 4096
P = 4
Q = 4
R = 512
CB = IN_F // Q        # 1024 input features per q block
OB = OUT_F // P       # 1024 output features per p block
N_TOK = 4 * 2048      # 8192 total tokens
N_CORE = N_TOK // N_CORES   # 1024 tokens per core

CHUNK = 512           # tokens per pipeline chunk
T2 = CB // 256        # 4 K-pairs per q in mm1 (256 rows per DoubleRow)
RT = R // 128         # 4 rank partition tiles
PR = RT // 2          # 2 rank pairs per p in mm2
FB = OB // 128        # 8 output-feature tiles per p in mm2

XS = 8.0              # host pre-scale of x
WS = 64.0             # host pre-scale of C and B
OS = 1.0 / (WS * WS)  # output drain un-scale

F32 = mybir.dt.float32
BF16 = mybir.dt.bfloat16
F8 = mybir.dt.float8e4
MULT = mybir.AluOpType.mult
ADD = mybir.AluOpType.add
SUB = mybir.AluOpType.subtract
IDENT = mybir.ActivationFunctionType.Identity
DR = mybir.MatmulPerfMode.DoubleRow

_cached_nc = None


def _build(n_core=N_CORE, chunk=CHUNK):
    nc = bacc.Bacc("TRN2", target_bir_lowering=False, debug=False,
                   enable_asserts=False)

    def din(name, shape, dtype):
        return nc.dram_tensor(name, shape, dtype, kind="ExternalInput").ap()

    xh8 = din("xh8", [IN_F, n_core], F8)
    xl8 = din("xl8", [IN_F, n_core], F8)
    ch8 = din("ch8", [IN_F, R], F8)
    cl8 = din("cl8", [IN_F, R], F8)
    bh8 = din("bh8", [P * R, OB], F8)
    bl8 = din("bl8", [P * R, OB], F8)
    dr = din("dr", [R, P * Q], F32)
    biasr = din("biasr", [128, OUT_F // 128], F32)
    out = nc.dram_tensor("out", [OUT_F, n_core], BF16,
                         kind="ExternalOutput").ap()

    n_chunks = n_core // chunk
    jB = min(1, n_chunks - 1)   # chunk whose mm1 streams in the B tiles

    with tile.TileContext(nc) as tc:
        with (
            tc.tile_pool(name="const", bufs=1) as cpool,
            tc.tile_pool(name="xp", bufs=2) as xpool,
            tc.tile_pool(name="zp", bufs=20) as zpool,
            tc.tile_pool(name="z8p", bufs=8 * min(2, n_chunks)) as z8pool,
            tc.tile_pool(name="outp", bufs=6) as outpool,
            tc.tile_pool(name="ybp", bufs=6) as ybpool,
            tc.tile_pool(name="yps", bufs=6, space="PSUM") as ypool,
            tc.tile_pool(name="ops", bufs=2, space="PSUM") as opool,
        ):
            # c*_sb[c_, t, i, r]: C^T rows c = t*256 + i*128 + c_
            # (t = q*T2 + t2; i = DoubleRow half)
            ch_sb = cpool.tile([128, Q * T2, 2, R], F8)
            cl_sb = cpool.tile([128, Q * T2, 2, R], F8)
            ch3 = ch8.rearrange("(t i p) r -> p t i r", i=2, p=128)
            cl3 = cl8.rearrange("(t i p) r -> p t i r", i=2, p=128)
            # b*_sb[c_, t, i, o]: B^T rows p_blk*R + pr*256 + i*128 + c_
            # (t = p_blk*PR + pr)
            bh_sb = cpool.tile([128, P * PR, 2, OB], F8)
            bl_sb = cpool.tile([128, P * PR, 2, OB], F8)
            bh3 = bh8.rearrange("(t i p) o -> p t i o", i=2, p=128)
            bl3 = bl8.rearrange("(t i p) o -> p t i o", i=2, p=128)
            # d_sb[p_, rt, p*Q + q] = D[p, q, rt*128 + p_] / XS / WS * WS
            d_sb = cpool.tile([128, RT, P * Q], F32)
            bias_sb = cpool.tile([128, OUT_F // 128], F32)

            z = {}
            z8 = {}
            x_sb = {}

            def emit_mm1(j):
                xh_t = xpool.tile([128, Q * T2, 2, chunk], F8,
                                  tag="xh", name=f"xh_{j}")
                xl_t = xpool.tile([128, Q * T2, 2, chunk], F8,
                                  tag="xl", name=f"xl_{j}")
                for q in range(Q):
                    ys = [
                        ypool.tile([128, chunk], F32, tag="y",
                                   name=f"y_{j}_{q}_{rt}")
                        for rt in range(RT)
                    ]
                    # stream this q's operands; hi parts first so the first
                    # (hh) pass can start before the lo parts land, and a
                    # tiny lead piece on the very first q
                    cs = slice(j * chunk, (j + 1) * chunk)

                    def xdma(src, dst, t0, w):
                        nc.sync.dma_start(
                            dst[:, t0:t0 + w, :, :],
                            src[t0 * 256:(t0 + w) * 256, cs]
                            .rearrange("(t i p) n -> p t i n", i=2, p=128))

                    def cdma(src3, dst, t0, w):
                        nc.sync.dma_start(dst[:, t0:t0 + w, :, :],
                                          src3[:, t0:t0 + w, :, :])

                    t0 = q * T2
                    pieces = ((t0, 1), (t0 + 1, T2 - 1)) if (j == 0 and
                                                             q == 0) \
                        else ((t0, T2),)
                    for pt, pw in pieces:
                        cdma(ch3, ch_sb, pt, pw) if j == 0 else None
                        xdma(xh8, xh_t, pt, pw)
                    if j == 0 and q == 0:
                        # tiny constants ride behind the first hi pieces
                        nc.sync.dma_start(
                            d_sb[:], dr.rearrange("(t p) s -> p t s", p=128))
                        nc.sync.dma_start(bias_sb[:], biasr[:, :])
                    if j == 0:
                        cdma(cl3, cl_sb, t0, T2)
                    xdma(xl8, xl_t, t0, T2)
                    if j == jB:
                        # one quarter of B rides behind each q's x
                        bs = slice(q * (P * PR // Q), (q + 1) * (P * PR // Q))
                        nc.sync.dma_start(bh_sb[:, bs, :, :], bh3[:, bs, :, :])
                        nc.sync.dma_start(bl_sb[:, bs, :, :], bl3[:, bs, :, :])
                    # pass-major hh, lh, hl: the hh pass only needs the hi
                    # operands that arrive first; xl is needed last
                    for wt, xt, first, last in (
                        (ch_sb, xh_t, True, False),
                        (cl_sb, xh_t, False, False),
                        (ch_sb, xl_t, False, True),
                    ):
                        for t2 in range(T2):
                            tg = q * T2 + t2
                            for rt in range(RT):
                                nc.tensor.matmul(
                                    ys[rt][:],
                                    lhsT=wt[:, tg, :,
                                            rt * 128:(rt + 1) * 128],
                                    rhs=xt[:, tg, :, :],
                                    start=(first and t2 == 0),
                                    stop=(last and t2 == T2 - 1),
                                    perf_mode=DR)
                    # mix: z'[p,rt] += (D/8)[p,q,rt-slice] * y[rt] in bf16.
                    # ACT drains the bank once; the multiplies run on DVE
                    # except q1 which goes to Pool (load balance); the final
                    # z' splits into fp8 hi (ACT cast) / lo (DVE subtract),
                    # p-major so mm2's first p block unblocks earliest.
                    ybs = []
                    for rt in range(RT):
                        yb = ybpool.tile([128, chunk], F32, tag="yb",
                                         name=f"yb_{j}_{q}_{rt}")
                        nc.scalar.copy(yb[:], ys[rt][:])
                        ybs.append(yb)
                    if q < Q - 1:
                        for rt in range(RT):
                            for p in range(P):
                                yb = ybs[rt]
                                dcol = d_sb[:, rt,
                                            p * Q + q:p * Q + q + 1]
                                if q == 0:
                                    zt = zpool.tile([128, chunk], BF16,
                                                    tag="z",
                                                    name=f"z_{j}_{p}_{rt}")
                                    z[(j, p, rt)] = zt
                                    nc.vector.tensor_scalar_mul(
                                        zt[:], yb[:], dcol)
                                else:
                                    zt = z[(j, p, rt)]
                                    nc.vector.scalar_tensor_tensor(
                                        zt[:], yb[:], dcol, zt[:],
                                        op0=MULT, op1=ADD)
                    else:
                        # last q: per p, all 4 final mixes first (dense on
                        # DVE), then the fp8 splits (zh on ACT; zl on DVE/
                        # Pool alternating) so p0's z8 lands earliest
                        for p in range(P):
                            for rt in range(RT):
                                dcol = d_sb[:, rt,
                                            p * Q + q:p * Q + q + 1]
                                zt = z[(j, p, rt)]
                                nc.vector.scalar_tensor_tensor(
                                    zt[:], ybs[rt][:], dcol, zt[:],
                                    op0=MULT, op1=ADD)
                            for pr in range(PR):
                                z8[(j, p, pr)] = (
                                    z8pool.tile([128, 2, chunk], F8,
                                                tag="zh",
                                                name=f"zh_{j}_{p}_{pr}"),
                                    z8pool.tile([128, 2, chunk], F8,
                                                tag="zl",
                                                name=f"zl_{j}_{p}_{pr}"))
                            # zl rides on Pool except where it gates the
                            # tail (last chunk's first p blocks feed the
                            # final mm2 with nothing left to overlap)
                            zl_eng = (nc.vector
                                      if j == n_chunks - 1 and p < 2
                                      else nc.gpsimd)
                            for rt in range(RT):
                                pr, i = rt // 2, rt % 2
                                zh_t, zl_t = z8[(j, p, pr)]
                                zt = z[(j, p, rt)]
                                nc.gpsimd.tensor_copy(zh_t[:, i, :], zt[:])
                                zl_eng.tensor_tensor(
                                    zl_t[:, i, :], zt[:], zh_t[:, i, :],
                                    op=SUB)
                    yield

            def emit_mm2(j):
                for p in range(P):
                    for fb in range(FB):
                        ops = opool.tile([128, chunk], F32, tag="o",
                                         name=f"o_{j}_{p}_{fb}")
                        fs = slice(fb * 128, (fb + 1) * 128)
                        # pass-major hh, lh, hl: the zl-dependent pass last
                        for w_sb, zi, first, last in (
                            (bh_sb, 0, True, False),
                            (bl_sb, 0, False, False),
                            (bh_sb, 1, False, True),
                        ):
                            for pr in range(PR):
                                tg = p * PR + pr
                                nc.tensor.matmul(
                                    ops[:], lhsT=w_sb[:, tg, :, fs],
                                    rhs=z8[(j, p, pr)][zi][:],
                                    start=(first and pr == 0),
                                    stop=(last and pr == PR - 1),
                                    perf_mode=DR)
                        ot = outpool.tile([128, chunk], BF16, tag="ot",
                                          name=f"ot_{j}_{p}_{fb}")
                        t = p * FB + fb
                        nc.scalar.activation(ot[:], ops[:], IDENT,
                                             bias=bias_sb[:, t:t + 1],
                                             scale=OS)
                        nc.sync.dma_start(
                            out[t * 128:(t + 1) * 128,
                                j * chunk:(j + 1) * chunk],
                            ot[:])
                    yield
                for p in range(P):
                    for pr in range(PR):
                        del z8[(j, p, pr)]
                    for rt in range(RT):
                        del z[(j, p, rt)]

            # software pipeline: mm1(j+1) q-blocks interleave with mm2(j)
            # p-blocks so DMA demand (mm1-heavy) and ACT drains (mm2-heavy)
            # spread evenly and the PE never waits on a whole phase.
            prev = None
            for j in range(n_chunks):
                for _ in emit_mm1(j):
                    if prev is not None:
                        next(prev, None)
                if prev is not None:
                    for _ in prev:
                        pass
                prev = emit_mm2(j)
            for _ in prev:
                pass

    nc.compile()
    return nc


def _f8_split(a, scale):
    import ml_dtypes
    F8NP = ml_dtypes.float8_e4m3
    s = np.asarray(a, dtype=np.float32) * scale
    hi = s.astype(F8NP)
    lo = (s - hi.astype(np.float32)).astype(F8NP)
    return np.ascontiguousarray(hi), np.ascontiguousarray(lo)


def _prep_in_maps(x, B, C, D, bias):
    x2 = np.asarray(x, dtype=np.float32).reshape(N_TOK, IN_F)
    CH, CL = _f8_split(
        np.asarray(C, dtype=np.float32).transpose(0, 2, 1).reshape(IN_F, R),
        WS)
    BH, BL = _f8_split(
        np.asarray(B, dtype=np.float32).transpose(0, 2, 1).reshape(P * R, OB),
        WS)
    DRm = np.ascontiguousarray(
        np.asarray(D, dtype=np.float32).transpose(2, 0, 1)
        .reshape(R, P * Q) / XS)
    BIASR = np.ascontiguousarray(
        np.asarray(bias, dtype=np.float32).reshape(OUT_F // 128, 128).T)

    in_maps = []
    for c in range(N_CORES):
        XH, XL = _f8_split(x2[c * N_CORE:(c + 1) * N_CORE].T, XS)
        in_maps.append({
            "xh8": XH, "xl8": XL, "ch8": CH, "cl8": CL,
            "bh8": BH, "bl8": BL, "dr": DRm, "biasr": BIASR,
        })
    return in_maps


def _run(in_maps, trace=False):
    global _cached_nc
    if _cached_nc is None:
        _cached_nc = _build()
    import time
    for attempt in range(3):
        try:
            return run_bass_kernel_spmd(
                _cached_nc, in_maps, list(range(N_CORES)), trace=trace)
        except Exception:
            # transient device errors (e.g. NRT_EXEC_UNIT_UNRECOVERABLE
            # from a previously wedged core) usually clear on retry
            if attempt == 2:
                raise
            time.sleep(5.0 * (attempt + 1))


def kernel(x, B, C, D, bias):
    lead = np.asarray(x).shape[:-1]
    res = _run(_prep_in_maps(x, B, C, D, bias))
    # per-core outputs are feature-major bf16 [OUT_F, n_core]; transpose back
    outs = [np.ascontiguousarray(res.results[c]["out"].T.astype(np.float32))
            for c in range(N_CORES)]
    return np.concatenate(outs, axis=0).reshape(*lead, OUT_F)


# revision 37
# speedup vs baseline: 3.5649x; 1.1426x over previous
"""Trainium2 Bass kernel for nn_BlastLinear (block low-rank linear layer).

Math (reference):
  y[q,n,r] = sum_c x[n, q*1024+c] * C[q,r,c]          (mm1, per input block q)
  z[p,n,r] = sum_q D[p,q,r] * y[q,n,r]                (tiny mix over q)
  o[p,n,j] = sum_r z[p,n,r] * B[p,j,r]                (mm2, per output block p)
  out[n, p*1024+j] = o[p,n,j] + bias[p*1024+j]

Sharding: pure data-parallel over the 8192 tokens -> 1024 tokens per core,
weights replicated, no collectives.

Precision: fp8(e4m3) DoubleRow matmuls, 3-term hi/lo per GEMM:
  A@X ~= Ah@Xh + Ah@Xl + Al@Xh     (drops Al@Xl ~ (2^-4)^2)
DoubleRow packs K=256 per instruction at 0.5 cycles/row, so the 3-term
scheme costs 0.75x a single bf16 pass while keeping ~0.3% per-stage rms
error (measured end-to-end ~3.3e-3 against the 2e-2 tolerance). e4m3
subnormals would wreck the small-magnitude weights, so the host pre-scales
x by 8 and C/B by 64 into the e4m3 normal range and the inverse scales
fold into the D mix constants (D/8) and the output drain (scale 1/4096).

Layout keeps [feature-or-rank on partitions, tokens moving] end to end:
  mm1:  psum y[rt] [128r x 512n] += 3-term DoubleRow over K-pairs  (PE)
  mix:  ACT drains each y bank to SBUF f32 once (fast PSUM-bank free),
        z'[p,rt] (bf16) accumulated in-place over q on DVE
  split: zh8 = e4m3(z'), zl8 = e4m3(z' - zh8) on GPSIMD (Pool), written
        into [128, 2, 512] pair tiles shaped for DoubleRow rhs
  mm2:  psum o[fb] [128o x 512n] += 3-term DoubleRow over rank pairs (PE)
  out:  ACT drain fused with 1/4096 un-scale + per-partition bias add,
        bf16 out DMA'd feature-major; the host transposes/upcasts.

Chunks of 512 tokens; emission order mm1(0), mm1(1), mm2(0), mm2(1) so
each chunk's mix/split tail hides under the other chunk's matmuls.
~24 MiB HBM traffic (~70 us) under ~86 us of PE time.
"""

import numpy as np

import concourse.mybir as mybir
import concourse.tile as tile
from concourse import bacc
from concourse.bass_utils import run_bass_kernel_spmd

N_CORES = 8
IN_F = 4096
OUT_F = 4096
P = 4
Q = 4
R = 512
CB = IN_F // Q        # 1024 input features per q block
OB = OUT_F // P       # 1024 output features per p block
N_TOK = 4 * 2048      # 8192 total tokens
N_CORE = N_TOK // N_CORES   # 1024 tokens per core

CHUNK = 512           # tokens per pipeline chunk
T2 = CB // 256        # 4 K-pairs per q in mm1 (256 rows per DoubleRow)
RT = R // 128         # 4 rank partition tiles
PR = RT // 2          # 2 rank pairs per p in mm2
FB = OB // 128        # 8 output-feature tiles per p in mm2

XS = 8.0              # host pre-scale of x
WS = 64.0             # host pre-scale of C and B
OS = 1.0 / (WS * WS)  # output drain un-scale

F32 = mybir.dt.float32
BF16 = mybir.dt.bfloat16
F8 = mybir.dt.float8e4
MULT = mybir.AluOpType.mult
ADD = mybir.AluOpType.add
SUB = mybir.AluOpType.subtract
IDENT = mybir.ActivationFunctionType.Identity
DR = mybir.MatmulPerfMode.DoubleRow

_cached_nc = None


def _build(n_core=N_CORE, chunk=CHUNK):
    nc = bacc.Bacc("TRN2", target_bir_lowering=False, debug=False,
                   enable_asserts=False)

    def din(name, shape, dtype):
        return nc.dram_tensor(name, shape, dtype, kind="ExternalInput").ap()

    xh8 = din("xh8", [IN_F, n_core], F8)
    xl8 = din("xl8", [IN_F, n_core], F8)
    ch8 = din("ch8", [IN_F, R], F8)
    cl8 = din("cl8", [IN_F, R], F8)
    bh8 = din("bh8", [P * R, OB], F8)
    bl8 = din("bl8", [P * R, OB], F8)
    dr = din("dr", [R, P * Q], F32)
    biasr = din("biasr", [128, OUT_F // 128], F32)
    out = nc.dram_tensor("out", [OUT_F, n_core], BF16,
                         kind="ExternalOutput").ap()

    n_chunks = n_core // chunk
    jB = min(1, n_chunks - 1)   # chunk whose mm1 streams in the B tiles

    with tile.TileContext(nc) as tc:
        with (
            tc.tile_pool(name="const", bufs=1) as cpool,
            tc.tile_pool(name="xp", bufs=2) as xpool,
            tc.tile_pool(name="zp", bufs=20) as zpool,
            tc.tile_pool(name="z8p", bufs=8 * min(2, n_chunks)) as z8pool,
            tc.tile_pool(name="outp", bufs=6) as outpool,
            tc.tile_pool(name="ybp", bufs=6) as ybpool,
            tc.tile_pool(name="tp", bufs=8) as tpool,
            tc.tile_pool(name="yps", bufs=6, space="PSUM") as ypool,
            tc.tile_pool(name="ops", bufs=2, space="PSUM") as opool,
        ):
            # c*_sb[c_, t, i, r]: C^T rows c = t*256 + i*128 + c_
            # (t = q*T2 + t2; i = DoubleRow half)
            ch_sb = cpool.tile([128, Q * T2, 2, R], F8)
            cl_sb = cpool.tile([128, Q * T2, 2, R], F8)
            ch3 = ch8.rearrange("(t i p) r -> p t i r", i=2, p=128)
            cl3 = cl8.rearrange("(t i p) r -> p t i r", i=2, p=128)
            # b*_sb[c_, t, i, o]: B^T rows p_blk*R + pr*256 + i*128 + c_
            # (t = p_blk*PR + pr)
            bh_sb = cpool.tile([128, P * PR, 2, OB], F8)
            bl_sb = cpool.tile([128, P * PR, 2, OB], F8)
            bh3 = bh8.rearrange("(t i p) o -> p t i o", i=2, p=128)
            bl3 = bl8.rearrange("(t i p) o -> p t i o", i=2, p=128)
            # d_sb[p_, rt, p*Q + q] = D[p, q, rt*128 + p_] / XS / WS * WS
            d_sb = cpool.tile([128, RT, P * Q], F32)
            bias_sb = cpool.tile([128, OUT_F // 128], F32)

            z = {}
            z8 = {}
            x_sb = {}

            def emit_mm1(j):
                xh_t = xpool.tile([128, Q * T2, 2, chunk], F8,
                                  tag="xh", name=f"xh_{j}")
                xl_t = xpool.tile([128, Q * T2, 2, chunk], F8,
                                  tag="xl", name=f"xl_{j}")
                for q in range(Q):
                    ys = [
                        ypool.tile([128, chunk], F32, tag="y",
                                   name=f"y_{j}_{q}_{rt}")
                        for rt in range(RT)
                    ]
                    # stream this q's operands; hi parts first so the first
                    # (hh) pass can start before the lo parts land, and a
                    # tiny lead piece on the very first q
                    cs = slice(j * chunk, (j + 1) * chunk)

                    def xdma(src, dst, t0, w):
                        nc.sync.dma_start(
                            dst[:, t0:t0 + w, :, :],
                            src[t0 * 256:(t0 + w) * 256, cs]
                            .rearrange("(t i p) n -> p t i n", i=2, p=128))

                    def cdma(src3, dst, t0, w):
                        nc.sync.dma_start(dst[:, t0:t0 + w, :, :],
                                          src3[:, t0:t0 + w, :, :])

                    t0 = q * T2
                    pieces = ((t0, 1), (t0 + 1, T2 - 1)) if (j == 0 and
                                                             q == 0) \
                        else ((t0, T2),)
                    for pt, pw in pieces:
                        cdma(ch3, ch_sb, pt, pw) if j == 0 else None
                        xdma(xh8, xh_t, pt, pw)
                    if j == 0 and q == 0:
                        # tiny constants ride behind the first hi pieces
                        nc.sync.dma_start(
                            d_sb[:], dr.rearrange("(t p) s -> p t s", p=128))
                        nc.sync.dma_start(bias_sb[:], biasr[:, :])
                    if j == 0:
                        cdma(cl3, cl_sb, t0, T2)
                    xdma(xl8, xl_t, t0, T2)
                    if j == jB:
                        # one quarter of B rides behind each q's x
                        bs = slice(q * (P * PR // Q), (q + 1) * (P * PR // Q))
                        nc.sync.dma_start(bh_sb[:, bs, :, :], bh3[:, bs, :, :])
                        nc.sync.dma_start(bl_sb[:, bs, :, :], bl3[:, bs, :, :])
                    # pass-major hh, lh, hl: the hh pass only needs the hi
                    # operands that arrive first; xl is needed last
                    for wt, xt, first, last in (
                        (ch_sb, xh_t, True, False),
                        (cl_sb, xh_t, False, False),
                        (ch_sb, xl_t, False, True),
                    ):
                        for t2 in range(T2):
                            tg = q * T2 + t2
                            for rt in range(RT):
                                nc.tensor.matmul(
                                    ys[rt][:],
                                    lhsT=wt[:, tg, :,
                                            rt * 128:(rt + 1) * 128],
                                    rhs=xt[:, tg, :, :],
                                    start=(first and t2 == 0),
                                    stop=(last and t2 == T2 - 1),
                                    perf_mode=DR)
                    # mix: z'[p,rt] += (D/8)[p,q,rt-slice] * y[rt] in bf16.
                    # ACT drains the bank to bf16 once; DVE then runs in its
                    # fast 2-byte modes: tensor_scalar partial (194ns) +
                    # tensor_tensor accumulate (327ns) instead of the slow
                    # scalar_tensor_tensor path (594ns).
                    ybs = []
                    for rt in range(RT):
                        yb = ybpool.tile([128, chunk], BF16, tag="yb",
                                         name=f"yb_{j}_{q}_{rt}")
                        nc.scalar.copy(yb[:], ys[rt][:])
                        ybs.append(yb)

                    def emit_mix(p, rt):
                        yb = ybs[rt]
                        dcol = d_sb[:, rt, p * Q + q:p * Q + q + 1]
                        if q == 0:
                            zt = zpool.tile([128, chunk], BF16, tag="z",
                                            name=f"z_{j}_{p}_{rt}")
                            z[(j, p, rt)] = zt
                            nc.vector.tensor_scalar_mul(zt[:], yb[:], dcol)
                        else:
                            zt = z[(j, p, rt)]
                            tt = tpool.tile([128, chunk], BF16, tag="t",
                                            name=f"t_{j}_{q}_{p}_{rt}")
                            nc.vector.tensor_scalar_mul(tt[:], yb[:], dcol)
                            nc.vector.tensor_tensor(
                                zt[:], zt[:], tt[:], op=ADD)

                    if q < Q - 1:
                        for rt in range(RT):
                            for p in range(P):
                                emit_mix(p, rt)
                    else:
                        # last q p-major so mm2's first p unblocks earliest;
                        # then the fp8 splits (zh on Pool; zl on DVE/Pool)
                        for p in range(P):
                            for rt in range(RT):
                                emit_mix(p, rt)
                            for pr in range(PR):
                                z8[(j, p, pr)] = (
                                    z8pool.tile([128, 2, chunk], F8,
                                                tag="zh",
                                                name=f"zh_{j}_{p}_{pr}"),
                                    z8pool.tile([128, 2, chunk], F8,
                                                tag="zl",
                                                name=f"zl_{j}_{p}_{pr}"))
                            # fp8 split per rank pair: pr0 on DVE, pr1 on
                            # Pool, so one p's four tiles emerge from two
                            # engines in parallel instead of one serial queue
                            for rt in range(RT):
                                pr, i = rt // 2, rt % 2
                                zh_t, zl_t = z8[(j, p, pr)]
                                zt = z[(j, p, rt)]
                                eng = nc.vector if pr == 0 else nc.gpsimd
                                eng.tensor_copy(zh_t[:, i, :], zt[:])
                                eng.tensor_tensor(
                                    zl_t[:, i, :], zt[:], zh_t[:, i, :],
                                    op=SUB)
                    yield

            def emit_mm2(j):
                # the last chunk's mm2 runs after all mm1 work, so the idle
                # y banks can serve as extra o banks (drains stop gating)
                op_pool = opool if j < n_chunks - 1 else ypool
                otag = "o" if j < n_chunks - 1 else "y"
                for p in range(P):
                    ot = None
                    for fb in range(FB):
                        ops = op_pool.tile([128, chunk], F32, tag=otag,
                                           name=f"o_{j}_{p}_{fb}")
                        fs = slice(fb * 128, (fb + 1) * 128)
                        # pass-major hh, lh, hl: the zl-dependent pass last
                        for w_sb, zi, first, last in (
                            (bh_sb, 0, True, False),
                            (bl_sb, 0, False, False),
                            (bh_sb, 1, False, True),
                        ):
                            for pr in range(PR):
                                tg = p * PR + pr
                                nc.tensor.matmul(
                                    ops[:], lhsT=w_sb[:, tg, :, fs],
                                    rhs=z8[(j, p, pr)][zi][:],
                                    start=(first and pr == 0),
                                    stop=(last and pr == PR - 1),
                                    perf_mode=DR)
                        # drains land in fb-pair tiles; one DMA per pair
                        # halves the HWDGE instruction tax on the out path
                        if fb % 2 == 0:
                            ot = outpool.tile([128, 2, chunk], BF16,
                                              tag="ot",
                                              name=f"ot_{j}_{p}_{fb}")
                        t = p * FB + fb
                        nc.scalar.activation(ot[:, fb % 2, :], ops[:], IDENT,
                                             bias=bias_sb[:, t:t + 1],
                                             scale=OS)
                        if fb % 2 == 1:
                            t0 = p * FB + fb - 1
                            nc.sync.dma_start(
                                out[t0 * 128:(t0 + 2) * 128,
                                    j * chunk:(j + 1) * chunk]
                                .rearrange("(i p) n -> p i n", i=2, p=128),
                                ot[:])
                    yield
                for p in range(P):
                    for pr in range(PR):
                        del z8[(j, p, pr)]
                    for rt in range(RT):
                        del z[(j, p, rt)]

            # software pipeline: mm1(j+1) q-blocks interleave with mm2(j)
            # p-blocks so DMA demand (mm1-heavy) and ACT drains (mm2-heavy)
            # spread evenly and the PE never waits on a whole phase.
            prev = None
            for j in range(n_chunks):
                for _ in emit_mm1(j):
                    if prev is not None:
                        next(prev, None)
                if prev is not None:
                    for _ in prev:
                        pass
                prev = emit_mm2(j)
            for _ in prev:
                pass

    nc.compile()
    return nc


def _f8_split(a, scale):
    import ml_dtypes
    F8NP = ml_dtypes.float8_e4m3
    s = np.asarray(a, dtype=np.float32) * scale
    hi = s.astype(F8NP)
    lo = (s - hi.astype(np.float32)).astype(F8NP)
    return np.ascontiguousarray(hi), np.ascontiguousarray(lo)


def _prep_in_maps(x, B, C, D, bias):
    x2 = np.asarray(x, dtype=np.float32).reshape(N_TOK, IN_F)
    CH, CL = _f8_split(
        np.asarray(C, dtype=np.float32).transpose(0, 2, 1).reshape(IN_F, R),
        WS)
    BH, BL = _f8_split(
        np.asarray(B, dtype=np.float32).transpose(0, 2, 1).reshape(P * R, OB),
        WS)
    DRm = np.ascontiguousarray(
        np.asarray(D, dtype=np.float32).transpose(2, 0, 1)
        .reshape(R, P * Q) / XS)
    BIASR = np.ascontiguousarray(
        np.asarray(bias, dtype=np.float32).reshape(OUT_F // 128, 128).T)

    in_maps = []
    for c in range(N_CORES):
        XH, XL = _f8_split(x2[c * N_CORE:(c + 1) * N_CORE].T, XS)
        in_maps.append({
            "xh8": XH, "xl8": XL, "ch8": CH, "cl8": CL,
            "bh8": BH, "bl8": BL, "dr": DRm, "biasr": BIASR,
        })
    return in_maps


def _run(in_maps, trace=False):
    global _cached_nc
    if _cached_nc is None:
        _cached_nc = _build()
    import time
    for attempt in range(3):
        try:
            return run_bass_kernel_spmd(
                _cached_nc, in_maps, list(range(N_CORES)), trace=trace)
        except Exception:
            # transient device errors (e.g. NRT_EXEC_UNIT_UNRECOVERABLE
            # from a previously wedged core) usually clear on retry
            if attempt == 2:
                raise
            time.sleep(5.0 * (attempt + 1))


def kernel(x, B, C, D, bias):
    lead = np.asarray(x).shape[:-1]
    res = _run(_prep_in_maps(x, B, C, D, bias))
    # per-core outputs are feature-major bf16 [OUT_F, n_core]; transpose back
    outs = [np.ascontiguousarray(res.results[c]["out"].T.astype(np.float32))
            for c in range(N_CORES)]
    return np.concatenate(outs, axis=0).reshape(*lead, OUT_F)


# revision 47
# speedup vs baseline: 3.6655x; 1.0282x over previous
"""Trainium2 Bass kernel for nn_BlastLinear (block low-rank linear layer).

Math (reference):
  y[q,n,r] = sum_c x[n, q*1024+c] * C[q,r,c]          (mm1, per input block q)
  z[p,n,r] = sum_q D[p,q,r] * y[q,n,r]                (tiny mix over q)
  o[p,n,j] = sum_r z[p,n,r] * B[p,j,r]                (mm2, per output block p)
  out[n, p*1024+j] = o[p,n,j] + bias[p*1024+j]

Sharding: pure data-parallel over the 8192 tokens -> 1024 tokens per core,
weights replicated, no collectives.

Precision: fp8(e4m3) DoubleRow matmuls, 3-term hi/lo per GEMM:
  A@X ~= Ah@Xh + Ah@Xl + Al@Xh     (drops Al@Xl ~ (2^-4)^2)
DoubleRow packs K=256 per instruction at 0.5 cycles/row, so the 3-term
scheme costs 0.75x a single bf16 pass while keeping ~0.3% per-stage rms
error (measured end-to-end ~3.3e-3 against the 2e-2 tolerance). e4m3
subnormals would wreck the small-magnitude weights, so the host pre-scales
x by 8 and C/B by 64 into the e4m3 normal range and the inverse scales
fold into the D mix constants (D/8) and the output drain (scale 1/4096).

Layout keeps [feature-or-rank on partitions, tokens moving] end to end:
  mm1:  psum y[rt] [128r x 512n] += 3-term DoubleRow over K-pairs  (PE)
  mix:  ACT drains each y bank to SBUF f32 once (fast PSUM-bank free),
        z'[p,rt] (bf16) accumulated in-place over q on DVE
  split: zh8 = e4m3(z'), zl8 = e4m3(z' - zh8) on GPSIMD (Pool), written
        into [128, 2, 512] pair tiles shaped for DoubleRow rhs
  mm2:  psum o[fb] [128o x 512n] += 3-term DoubleRow over rank pairs (PE)
  out:  ACT drain fused with 1/4096 un-scale + per-partition bias add,
        bf16 out DMA'd feature-major; the host transposes/upcasts.

Chunks of 512 tokens; emission order mm1(0), mm1(1), mm2(0), mm2(1) so
each chunk's mix/split tail hides under the other chunk's matmuls.
~24 MiB HBM traffic (~70 us) under ~86 us of PE time.
"""

import numpy as np

import concourse.mybir as mybir
import concourse.tile as tile
from concourse import bacc
from concourse.bass_utils import run_bass_kernel_spmd

N_CORES = 8
IN_F = 4096
OUT_F = 4096
P = 4
Q = 4
R = 512
CB = IN_F // Q        # 1024 input features per q block
OB = OUT_F // P       # 1024 output features per p block
N_TOK = 4 * 2048      # 8192 total tokens
N_CORE = N_TOK // N_CORES   # 1024 tokens per core

CHUNK = 512           # tokens per pipeline chunk
T2 = CB // 256        # 4 K-pairs per q in mm1 (256 rows per DoubleRow)
RT = R // 128         # 4 rank partition tiles
PR = RT // 2          # 2 rank pairs per p in mm2
FB = OB // 128        # 8 output-feature tiles per p in mm2

XS = 8.0              # host pre-scale of x
WS = 64.0             # host pre-scale of C and B
OS = 1.0 / (WS * WS)  # output drain un-scale

F32 = mybir.dt.float32
BF16 = mybir.dt.bfloat16
F8 = mybir.dt.float8e4
MULT = mybir.AluOpType.mult
ADD = mybir.AluOpType.add
SUB = mybir.AluOpType.subtract
IDENT = mybir.ActivationFunctionType.Identity
DR = mybir.MatmulPerfMode.DoubleRow

_cached_nc = None


def _build(n_core=N_CORE, chunk=CHUNK):
    nc = bacc.Bacc("TRN2", target_bir_lowering=False, debug=False,
                   enable_asserts=False)

    def din(name, shape, dtype):
        return nc.dram_tensor(name, shape, dtype, kind="ExternalInput").ap()

    xh8 = din("xh8", [IN_F, n_core], F8)
    xl8 = din("xl8", [IN_F, n_core], F8)
    ch8 = din("ch8", [IN_F, R], F8)
    cl8 = din("cl8", [IN_F, R], F8)
    bh8 = din("bh8", [P * R, OB], F8)
    bl8 = din("bl8", [P * R, OB], F8)
    dr = din("dr", [R, P * Q], F32)
    biasr = din("biasr", [128, OUT_F // 128], F32)
    out = nc.dram_tensor("out", [OUT_F, n_core], BF16,
                         kind="ExternalOutput").ap()

    n_chunks = n_core // chunk
    jB = min(1, n_chunks - 1)   # chunk whose mm1 streams in the B tiles

    with tile.TileContext(nc) as tc:
        with (
            tc.tile_pool(name="const", bufs=1) as cpool,
            tc.tile_pool(name="xp", bufs=2) as xpool,
            tc.tile_pool(name="zp", bufs=20) as zpool,
            tc.tile_pool(name="z8p", bufs=8 * min(2, n_chunks)) as z8pool,
            tc.tile_pool(name="outp", bufs=6) as outpool,
            tc.tile_pool(name="ybp", bufs=6) as ybpool,
            tc.tile_pool(name="tp", bufs=8) as tpool,
            tc.tile_pool(name="yps", bufs=6, space="PSUM") as ypool,
            tc.tile_pool(name="ops", bufs=2, space="PSUM") as opool,
        ):
            # c*_sb[c_, t, i, r]: C^T rows c = t*256 + i*128 + c_
            # (t = q*T2 + t2; i = DoubleRow half)
            ch_sb = cpool.tile([128, Q * T2, 2, R], F8)
            cl_sb = cpool.tile([128, Q * T2, 2, R], F8)
            ch3 = ch8.rearrange("(t i p) r -> p t i r", i=2, p=128)
            cl3 = cl8.rearrange("(t i p) r -> p t i r", i=2, p=128)
            # b*_sb[c_, t, i, o]: B^T rows p_blk*R + pr*256 + i*128 + c_
            # (t = p_blk*PR + pr)
            bh_sb = cpool.tile([128, P * PR, 2, OB], F8)
            bl_sb = cpool.tile([128, P * PR, 2, OB], F8)
            bh3 = bh8.rearrange("(t i p) o -> p t i o", i=2, p=128)
            bl3 = bl8.rearrange("(t i p) o -> p t i o", i=2, p=128)
            # d_sb[p_, rt, p*Q + q] = D[p, q, rt*128 + p_] / XS / WS * WS
            d_sb = cpool.tile([128, RT, P * Q], F32)
            bias_sb = cpool.tile([128, OUT_F // 128], F32)

            z = {}
            z8 = {}
            x_sb = {}

            def emit_mm1(j):
                xh_t = xpool.tile([128, Q * T2, 2, chunk], F8,
                                  tag="xh", name=f"xh_{j}")
                xl_t = xpool.tile([128, Q * T2, 2, chunk], F8,
                                  tag="xl", name=f"xl_{j}")
                for q in range(Q):
                    ys = [
                        ypool.tile([128, chunk], F32, tag="y",
                                   name=f"y_{j}_{q}_{rt}")
                        for rt in range(RT)
                    ]
                    # stream this q's operands; hi parts first so the first
                    # (hh) pass can start before the lo parts land, and a
                    # tiny lead piece on the very first q
                    cs = slice(j * chunk, (j + 1) * chunk)

                    def xdma(src, dst, t0, w):
                        nc.sync.dma_start(
                            dst[:, t0:t0 + w, :, :],
                            src[t0 * 256:(t0 + w) * 256, cs]
                            .rearrange("(t i p) n -> p t i n", i=2, p=128))

                    def cdma(src3, dst, t0, w):
                        nc.sync.dma_start(dst[:, t0:t0 + w, :, :],
                                          src3[:, t0:t0 + w, :, :])

                    t0 = q * T2
                    pieces = ((t0, 1), (t0 + 1, T2 - 1)) if (j == 0 and
                                                             q == 0) \
                        else ((t0, T2),)
                    for pt, pw in pieces:
                        cdma(ch3, ch_sb, pt, pw) if j == 0 else None
                        xdma(xh8, xh_t, pt, pw)
                    if j == 0 and q == 0:
                        # tiny constants ride behind the first hi pieces
                        nc.sync.dma_start(
                            d_sb[:], dr.rearrange("(t p) s -> p t s", p=128))
                        nc.sync.dma_start(bias_sb[:], biasr[:, :])
                    if j == 0:
                        cdma(cl3, cl_sb, t0, T2)
                    xdma(xl8, xl_t, t0, T2)
                    if j == jB:
                        # one quarter of B rides behind each q's x
                        bs = slice(q * (P * PR // Q), (q + 1) * (P * PR // Q))
                        nc.sync.dma_start(bh_sb[:, bs, :, :], bh3[:, bs, :, :])
                        nc.sync.dma_start(bl_sb[:, bs, :, :], bl3[:, bs, :, :])
                    # pass-major hh, lh, hl: the hh pass only needs the hi
                    # operands that arrive first; xl is needed last
                    for wt, xt, first, last in (
                        (ch_sb, xh_t, True, False),
                        (cl_sb, xh_t, False, False),
                        (ch_sb, xl_t, False, True),
                    ):
                        for t2 in range(T2):
                            tg = q * T2 + t2
                            for rt in range(RT):
                                nc.tensor.matmul(
                                    ys[rt][:],
                                    lhsT=wt[:, tg, :,
                                            rt * 128:(rt + 1) * 128],
                                    rhs=xt[:, tg, :, :],
                                    start=(first and t2 == 0),
                                    stop=(last and t2 == T2 - 1),
                                    perf_mode=DR)
                    # mix: z'[p,rt] += (D/8)[p,q,rt-slice] * y[rt] in bf16.
                    # ACT drains the bank to bf16 once; DVE then runs in its
                    # fast 2-byte modes: tensor_scalar partial (194ns) +
                    # tensor_tensor accumulate (327ns) instead of the slow
                    # scalar_tensor_tensor path (594ns).
                    ybs = []
                    for rt in range(RT):
                        yb = ybpool.tile([128, chunk], BF16, tag="yb",
                                         name=f"yb_{j}_{q}_{rt}")
                        nc.scalar.copy(yb[:], ys[rt][:])
                        ybs.append(yb)

                    def emit_mix(p, rt):
                        yb = ybs[rt]
                        dcol = d_sb[:, rt, p * Q + q:p * Q + q + 1]
                        if q == 0:
                            zt = zpool.tile([128, chunk], BF16, tag="z",
                                            name=f"z_{j}_{p}_{rt}")
                            z[(j, p, rt)] = zt
                            nc.vector.tensor_scalar_mul(zt[:], yb[:], dcol)
                        else:
                            zt = z[(j, p, rt)]
                            tt = tpool.tile([128, chunk], BF16, tag="t",
                                            name=f"t_{j}_{q}_{p}_{rt}")
                            nc.vector.tensor_scalar_mul(tt[:], yb[:], dcol)
                            nc.vector.tensor_tensor(
                                zt[:], zt[:], tt[:], op=ADD)

                    if q < Q - 1:
                        for rt in range(RT):
                            for p in range(P):
                                emit_mix(p, rt)
                    else:
                        # last q p-major so mm2's first p unblocks earliest;
                        # then the fp8 splits (zh on Pool; zl on DVE/Pool)
                        for p in range(P):
                            for rt in range(RT):
                                emit_mix(p, rt)
                            for pr in range(PR):
                                z8[(j, p, pr)] = (
                                    z8pool.tile([128, 2, chunk], F8,
                                                tag="zh",
                                                name=f"zh_{j}_{p}_{pr}"),
                                    z8pool.tile([128, 2, chunk], F8,
                                                tag="zl",
                                                name=f"zl_{j}_{p}_{pr}"))
                            # fp8 split per rank pair: pr0 on DVE, pr1 on
                            # Pool, so one p's four tiles emerge from two
                            # engines in parallel instead of one serial queue
                            for rt in range(RT):
                                pr, i = rt // 2, rt % 2
                                zh_t, zl_t = z8[(j, p, pr)]
                                zt = z[(j, p, rt)]
                                eng = nc.vector if pr == 0 else nc.gpsimd
                                eng.tensor_copy(zh_t[:, i, :], zt[:])
                                eng.tensor_tensor(
                                    zl_t[:, i, :], zt[:], zh_t[:, i, :],
                                    op=SUB)
                    yield

            def emit_mm2(j):
                # the last chunk's mm2 runs after all mm1 work, so the idle
                # y banks can serve as extra o banks (drains stop gating)
                for p in range(P):
                    # p-blocks that run while the next chunk's mm1 still
                    # owns the y banks use the dedicated o banks; trailing
                    # blocks (and the whole last chunk) borrow freed y banks
                    late = j == n_chunks - 1 or p >= 2
                    op_pool = ypool if late else opool
                    otag = "y" if late else "o"
                    ot = None
                    for fb in range(FB):
                        ops = op_pool.tile([128, chunk], F32, tag=otag,
                                           name=f"o_{j}_{p}_{fb}")
                        fs = slice(fb * 128, (fb + 1) * 128)
                        # pass-major hh, lh, hl: the zl-dependent pass last
                        for w_sb, zi, first, last in (
                            (bh_sb, 0, True, False),
                            (bl_sb, 0, False, False),
                            (bh_sb, 1, False, True),
                        ):
                            for pr in range(PR):
                                tg = p * PR + pr
                                nc.tensor.matmul(
                                    ops[:], lhsT=w_sb[:, tg, :, fs],
                                    rhs=z8[(j, p, pr)][zi][:],
                                    start=(first and pr == 0),
                                    stop=(last and pr == PR - 1),
                                    perf_mode=DR)
                        # drains land in fb-pair tiles; one DMA per pair
                        # halves the HWDGE instruction tax on the out path.
                        # The very last p-block instead uses single-tile
                        # DMAs and alternates drains onto the (by then
                        # idle) DVE so the end-of-kernel chain is short.
                        t = p * FB + fb
                        if fb % 2 == 0:
                            ot = outpool.tile([128, 2, chunk], BF16,
                                              tag="ot",
                                              name=f"ot_{j}_{p}_{fb}")
                        nc.scalar.activation(ot[:, fb % 2, :], ops[:], IDENT,
                                             bias=bias_sb[:, t:t + 1],
                                             scale=OS)
                        last_pair = (j == n_chunks - 1 and p == P - 1
                                     and fb >= FB - 2)
                        if last_pair:
                            # ship each half as its own DMA so the kernel's
                            # closing chain rides on a 128KB transfer
                            nc.sync.dma_start(
                                out[t * 128:(t + 1) * 128,
                                    j * chunk:(j + 1) * chunk],
                                ot[:, fb % 2, :])
                        elif fb % 2 == 1:
                            t0 = p * FB + fb - 1
                            nc.sync.dma_start(
                                out[t0 * 128:(t0 + 2) * 128,
                                    j * chunk:(j + 1) * chunk]
                                .rearrange("(i p) n -> p i n", i=2, p=128),
                                ot[:])
                    yield
                for p in range(P):
                    for pr in range(PR):
                        del z8[(j, p, pr)]
                    for rt in range(RT):
                        del z[(j, p, rt)]

            # software pipeline: mm1(j+1) q-blocks interleave with mm2(j)
            # p-blocks so DMA demand (mm1-heavy) and ACT drains (mm2-heavy)
            # spread evenly and the PE never waits on a whole phase. The
            # two-block lead gives each chunk's mix/split chain time to
            # produce z8 before mm2 wants it, and leaves mm2 p-blocks after
            # the last mm1 to hide the final chunk's chain.
            prev = None
            for j in range(n_chunks):
                k = 0
                for _ in emit_mm1(j):
                    if prev is not None and k >= 1:
                        next(prev, None)
                    k += 1
                if prev is not None:
                    for _ in prev:
                        pass
                prev = emit_mm2(j)
            for _ in prev:
                pass

    nc.compile()
    return nc


def _f8_split(a, scale):
    import ml_dtypes
    F8NP = ml_dtypes.float8_e4m3
    s = np.asarray(a, dtype=np.float32) * scale
    hi = s.astype(F8NP)
    lo = (s - hi.astype(np.float32)).astype(F8NP)
    return np.ascontiguousarray(hi), np.ascontiguousarray(lo)


def _prep_in_maps(x, B, C, D, bias):
    x2 = np.asarray(x, dtype=np.float32).reshape(N_TOK, IN_F)
    CH, CL = _f8_split(
        np.asarray(C, dtype=np.float32).transpose(0, 2, 1).reshape(IN_F, R),
        WS)
    BH, BL = _f8_split(
        np.asarray(B, dtype=np.float32).transpose(0, 2, 1).reshape(P * R, OB),
        WS)
    DRm = np.ascontiguousarray(
        np.asarray(D, dtype=np.float32).transpose(2, 0, 1)
        .reshape(R, P * Q) / XS)
    BIASR = np.ascontiguousarray(
        np.asarray(bias, dtype=np.float32).reshape(OUT_F // 128, 128).T)

    in_maps = []
    for c in range(N_CORES):
        XH, XL = _f8_split(x2[c * N_CORE:(c + 1) * N_CORE].T, XS)
        in_maps.append({
            "xh8": XH, "xl8": XL, "ch8": CH, "cl8": CL,
            "bh8": BH, "bl8": BL, "dr": DRm, "biasr": BIASR,
        })
    return in_maps


def _run(in_maps, trace=False):
    global _cached_nc
    if _cached_nc is None:
        _cached_nc = _build()
    import time
    for attempt in range(3):
        try:
            return run_bass_kernel_spmd(
                _cached_nc, in_maps, list(range(N_CORES)), trace=trace)
        except Exception:
            # transient device errors (e.g. NRT_EXEC_UNIT_UNRECOVERABLE
            # from a previously wedged core) usually clear on retry
            if attempt == 2:
                raise
            time.sleep(5.0 * (attempt + 1))


def kernel(x, B, C, D, bias):
    lead = np.asarray(x).shape[:-1]
    res = _run(_prep_in_maps(x, B, C, D, bias))
    # per-core outputs are feature-major bf16 [OUT_F, n_core]; transpose back
    outs = [np.ascontiguousarray(res.results[c]["out"].T.astype(np.float32))
            for c in range(N_CORES)]
    return np.concatenate(outs, axis=0).reshape(*lead, OUT_F)


# revision 53
# speedup vs baseline: 3.6729x; 1.0020x over previous
"""Trainium2 Bass kernel for nn_BlastLinear (block low-rank linear layer).

Math (reference):
  y[q,n,r] = sum_c x[n, q*1024+c] * C[q,r,c]          (mm1, per input block q)
  z[p,n,r] = sum_q D[p,q,r] * y[q,n,r]                (tiny mix over q)
  o[p,n,j] = sum_r z[p,n,r] * B[p,j,r]                (mm2, per output block p)
  out[n, p*1024+j] = o[p,n,j] + bias[p*1024+j]

Sharding: pure data-parallel over the 8192 tokens -> 1024 tokens per core,
weights replicated, no collectives.

Precision: fp8(e4m3) DoubleRow matmuls, 3-term hi/lo per GEMM:
  A@X ~= Ah@Xh + Ah@Xl + Al@Xh     (drops Al@Xl ~ (2^-4)^2)
DoubleRow packs K=256 per instruction at 0.5 cycles/row, so the 3-term
scheme costs 0.75x a single bf16 pass while keeping ~0.3% per-stage rms
error (measured end-to-end ~3.3e-3 against the 2e-2 tolerance). e4m3
subnormals would wreck the small-magnitude weights, so the host pre-scales
x by 8 and C/B by 64 into the e4m3 normal range and the inverse scales
fold into the D mix constants (D/8) and the output drain (scale 1/4096).

Layout keeps [feature-or-rank on partitions, tokens moving] end to end:
  mm1:  psum y[rt] [128r x 512n] += 3-term DoubleRow over K-pairs  (PE)
  mix:  ACT drains each y bank to SBUF bf16 once (fast PSUM-bank free),
        then z'[p,rt] accumulates over q on DVE via its fast 2-byte
        modes: tensor_scalar partial (194ns) + tensor_tensor add (327ns)
  split: zh8 = e4m3(z'), zl8 = e4m3(z' - zh8), written into [128, 2, 512]
        pair tiles shaped as DoubleRow rhs; rank-pair 0 on DVE, pair 1 on
        GPSIMD so each p's four tiles emerge from two engines in parallel
  mm2:  psum o[fb] [128o x 512n] += 3-term DoubleRow over rank pairs (PE)
  out:  ACT drain fused with 1/4096 un-scale + per-partition bias add,
        bf16 fb-pair tiles DMA'd feature-major; the host transposes.

Chunks of 512 tokens, software-pipelined with a two-block lead: PE order
is mm1(0), then mm1(j+1) q-blocks interleaved with mm2(j) p-blocks, so
each chunk's mix/split chain produces z8 while the PE chews the other
chunk, and DMA demand (mm1-heavy) spreads across the whole timeline.
mm2 p-blocks that run after their neighbour chunk's mm1 finished borrow
the idle y PSUM banks so ACT drains never gate the 2-bank o pool.
~24 MiB HBM traffic (~70 us) under ~82 us of PE matmul time; TimelineSim
99.6 us vs the 365.8 us 3-pass-f32r baseline.
"""

import numpy as np

import concourse.mybir as mybir
import concourse.tile as tile
from concourse import bacc
from concourse.bass_utils import run_bass_kernel_spmd

N_CORES = 8
IN_F = 4096
OUT_F = 4096
P = 4
Q = 4
R = 512
CB = IN_F // Q        # 1024 input features per q block
OB = OUT_F // P       # 1024 output features per p block
N_TOK = 4 * 2048      # 8192 total tokens
N_CORE = N_TOK // N_CORES   # 1024 tokens per core

CHUNK = 512           # tokens per pipeline chunk
T2 = CB // 256        # 4 K-pairs per q in mm1 (256 rows per DoubleRow)
RT = R // 128         # 4 rank partition tiles
PR = RT // 2          # 2 rank pairs per p in mm2
FB = OB // 128        # 8 output-feature tiles per p in mm2

XS = 8.0              # host pre-scale of x
WS = 64.0             # host pre-scale of C and B
OS = 1.0 / (WS * WS)  # output drain un-scale

F32 = mybir.dt.float32
BF16 = mybir.dt.bfloat16
F8 = mybir.dt.float8e4
MULT = mybir.AluOpType.mult
ADD = mybir.AluOpType.add
SUB = mybir.AluOpType.subtract
IDENT = mybir.ActivationFunctionType.Identity
DR = mybir.MatmulPerfMode.DoubleRow

_cached_nc = None


def _build(n_core=N_CORE, chunk=CHUNK):
    nc = bacc.Bacc("TRN2", target_bir_lowering=False, debug=False,
                   enable_asserts=False)

    def din(name, shape, dtype):
        return nc.dram_tensor(name, shape, dtype, kind="ExternalInput").ap()

    xh8 = din("xh8", [IN_F, n_core], F8)
    xl8 = din("xl8", [IN_F, n_core], F8)
    ch8 = din("ch8", [IN_F, R], F8)
    cl8 = din("cl8", [IN_F, R], F8)
    bh8 = din("bh8", [P * R, OB], F8)
    bl8 = din("bl8", [P * R, OB], F8)
    dr = din("dr", [R, P * Q], F32)
    biasr = din("biasr", [128, OUT_F // 128], F32)
    out = nc.dram_tensor("out", [OUT_F, n_core], BF16,
                         kind="ExternalOutput").ap()

    n_chunks = n_core // chunk
    jB = min(1, n_chunks - 1)   # chunk whose mm1 streams in the B tiles

    with tile.TileContext(nc) as tc:
        with (
            tc.tile_pool(name="const", bufs=1) as cpool,
            tc.tile_pool(name="xp", bufs=2) as xpool,
            tc.tile_pool(name="zp", bufs=20) as zpool,
            tc.tile_pool(name="z8p", bufs=8 * min(2, n_chunks)) as z8pool,
            tc.tile_pool(name="outp", bufs=6) as outpool,
            tc.tile_pool(name="ybp", bufs=6) as ybpool,
            tc.tile_pool(name="tp", bufs=8) as tpool,
            tc.tile_pool(name="yps", bufs=6, space="PSUM") as ypool,
            tc.tile_pool(name="ops", bufs=2, space="PSUM") as opool,
        ):
            # c*_sb[c_, t, i, r]: C^T rows c = t*256 + i*128 + c_
            # (t = q*T2 + t2; i = DoubleRow half)
            ch_sb = cpool.tile([128, Q * T2, 2, R], F8)
            cl_sb = cpool.tile([128, Q * T2, 2, R], F8)
            ch3 = ch8.rearrange("(t i p) r -> p t i r", i=2, p=128)
            cl3 = cl8.rearrange("(t i p) r -> p t i r", i=2, p=128)
            # b*_sb[c_, t, i, o]: B^T rows p_blk*R + pr*256 + i*128 + c_
            # (t = p_blk*PR + pr)
            bh_sb = cpool.tile([128, P * PR, 2, OB], F8)
            bl_sb = cpool.tile([128, P * PR, 2, OB], F8)
            bh3 = bh8.rearrange("(t i p) o -> p t i o", i=2, p=128)
            bl3 = bl8.rearrange("(t i p) o -> p t i o", i=2, p=128)
            # d_sb[p_, rt, p*Q + q] = D[p, q, rt*128 + p_] / XS / WS * WS
            d_sb = cpool.tile([128, RT, P * Q], F32)
            bias_sb = cpool.tile([128, OUT_F // 128], F32)

            z = {}
            z8 = {}

            def emit_mm1(j):
                xh_t = xpool.tile([128, Q * T2, 2, chunk], F8,
                                  tag="xh", name=f"xh_{j}")
                xl_t = xpool.tile([128, Q * T2, 2, chunk], F8,
                                  tag="xl", name=f"xl_{j}")
                for q in range(Q):
                    ys = [
                        ypool.tile([128, chunk], F32, tag="y",
                                   name=f"y_{j}_{q}_{rt}")
                        for rt in range(RT)
                    ]
                    # stream this q's operands; hi parts first so the first
                    # (hh) pass can start before the lo parts land, and a
                    # tiny lead piece on the very first q
                    cs = slice(j * chunk, (j + 1) * chunk)

                    def xdma(src, dst, t0, w):
                        nc.sync.dma_start(
                            dst[:, t0:t0 + w, :, :],
                            src[t0 * 256:(t0 + w) * 256, cs]
                            .rearrange("(t i p) n -> p t i n", i=2, p=128))

                    def cdma(src3, dst, t0, w):
                        nc.sync.dma_start(dst[:, t0:t0 + w, :, :],
                                          src3[:, t0:t0 + w, :, :])

                    t0 = q * T2
                    pieces = ((t0, 1), (t0 + 1, T2 - 1)) if (j == 0 and
                                                             q == 0) \
                        else ((t0, T2),)
                    for pt, pw in pieces:
                        cdma(ch3, ch_sb, pt, pw) if j == 0 else None
                        xdma(xh8, xh_t, pt, pw)
                    lo_pieces = ((t0, 2), (t0 + 2, T2 - 2)) \
                        if (j == 0 and q == 0) else ((t0, T2),)
                    for pt, pw in lo_pieces:
                        if j == 0:
                            cdma(cl3, cl_sb, pt, pw)
                        xdma(xl8, xl_t, pt, pw)
                    if j == 0 and q == 0:
                        # tiny constants ride behind q0's pieces; the mix
                        # only needs them ~8us in
                        nc.sync.dma_start(
                            d_sb[:], dr.rearrange("(t p) s -> p t s", p=128))
                        nc.sync.dma_start(bias_sb[:], biasr[:, :])
                    if j == jB and q < 2:
                        # first two B quarters ride behind q0/q1's x; the
                        # rest defer to mm2's early p-blocks (the interleave
                        # window is DMA-saturated, the mm2 tail is not)
                        bs = slice(q * 2, (q + 1) * 2)
                        nc.sync.dma_start(bh_sb[:, bs, :, :], bh3[:, bs, :, :])
                        nc.sync.dma_start(bl_sb[:, bs, :, :], bl3[:, bs, :, :])
                    # pass-major hh, lh, hl: the hh pass only needs the hi
                    # operands that arrive first; xl is needed last
                    for wt, xt, first, last in (
                        (ch_sb, xh_t, True, False),
                        (cl_sb, xh_t, False, False),
                        (ch_sb, xl_t, False, True),
                    ):
                        for t2 in range(T2):
                            tg = q * T2 + t2
                            for rt in range(RT):
                                nc.tensor.matmul(
                                    ys[rt][:],
                                    lhsT=wt[:, tg, :,
                                            rt * 128:(rt + 1) * 128],
                                    rhs=xt[:, tg, :, :],
                                    start=(first and t2 == 0),
                                    stop=(last and t2 == T2 - 1),
                                    perf_mode=DR)
                    # mix: z'[p,rt] += (D/8)[p,q,rt-slice] * y[rt] in bf16.
                    # ACT drains the bank to bf16 once; DVE then runs in its
                    # fast 2-byte modes: tensor_scalar partial (194ns) +
                    # tensor_tensor accumulate (327ns) instead of the slow
                    # scalar_tensor_tensor path (594ns).
                    ybs = []
                    for rt in range(RT):
                        yb = ybpool.tile([128, chunk], BF16, tag="yb",
                                         name=f"yb_{j}_{q}_{rt}")
                        nc.scalar.copy(yb[:], ys[rt][:])
                        ybs.append(yb)

                    def emit_mix(p, rt):
                        yb = ybs[rt]
                        dcol = d_sb[:, rt, p * Q + q:p * Q + q + 1]
                        if q == 0:
                            zt = zpool.tile([128, chunk], BF16, tag="z",
                                            name=f"z_{j}_{p}_{rt}")
                            z[(j, p, rt)] = zt
                            nc.vector.tensor_scalar_mul(zt[:], yb[:], dcol)
                        else:
                            zt = z[(j, p, rt)]
                            tt = tpool.tile([128, chunk], BF16, tag="t",
                                            name=f"t_{j}_{q}_{p}_{rt}")
                            nc.vector.tensor_scalar_mul(tt[:], yb[:], dcol)
                            nc.vector.tensor_tensor(
                                zt[:], zt[:], tt[:], op=ADD)

                    if q < Q - 1:
                        for rt in range(RT):
                            for p in range(P):
                                emit_mix(p, rt)
                    else:
                        # last q p-major so mm2's first p unblocks earliest;
                        # then the fp8 splits (zh on Pool; zl on DVE/Pool)
                        for p in range(P):
                            for rt in range(RT):
                                emit_mix(p, rt)
                            for pr in range(PR):
                                z8[(j, p, pr)] = (
                                    z8pool.tile([128, 2, chunk], F8,
                                                tag="zh",
                                                name=f"zh_{j}_{p}_{pr}"),
                                    z8pool.tile([128, 2, chunk], F8,
                                                tag="zl",
                                                name=f"zl_{j}_{p}_{pr}"))
                            # fp8 split per rank pair: pr0 on DVE, pr1 on
                            # Pool, so one p's four tiles emerge from two
                            # engines in parallel instead of one serial queue
                            for rt in range(RT):
                                pr, i = rt // 2, rt % 2
                                zh_t, zl_t = z8[(j, p, pr)]
                                zt = z[(j, p, rt)]
                                eng = nc.vector if pr == 0 else nc.gpsimd
                                eng.tensor_copy(zh_t[:, i, :], zt[:])
                                eng.tensor_tensor(
                                    zl_t[:, i, :], zt[:], zh_t[:, i, :],
                                    op=SUB)
                    yield

            def emit_mm2(j):
                # the last chunk's mm2 runs after all mm1 work, so the idle
                # y banks can serve as extra o banks (drains stop gating)
                for p in range(P):
                    if j == 0 and p < 2:
                        bs = slice((p + 2) * 2, (p + 3) * 2)
                        nc.sync.dma_start(bh_sb[:, bs, :, :],
                                          bh3[:, bs, :, :])
                        nc.sync.dma_start(bl_sb[:, bs, :, :],
                                          bl3[:, bs, :, :])
                    # p-blocks that run while the next chunk's mm1 still
                    # owns the y banks use the dedicated o banks; trailing
                    # blocks (and the whole last chunk) borrow freed y banks
                    late = j == n_chunks - 1 or p >= 2
                    op_pool = ypool if late else opool
                    otag = "y" if late else "o"
                    ot = None
                    for fb in range(FB):
                        ops = op_pool.tile([128, chunk], F32, tag=otag,
                                           name=f"o_{j}_{p}_{fb}")
                        fs = slice(fb * 128, (fb + 1) * 128)
                        # pass-major hh, lh, hl: the zl-dependent pass last
                        for w_sb, zi, first, last in (
                            (bh_sb, 0, True, False),
                            (bl_sb, 0, False, False),
                            (bh_sb, 1, False, True),
                        ):
                            for pr in range(PR):
                                tg = p * PR + pr
                                nc.tensor.matmul(
                                    ops[:], lhsT=w_sb[:, tg, :, fs],
                                    rhs=z8[(j, p, pr)][zi][:],
                                    start=(first and pr == 0),
                                    stop=(last and pr == PR - 1),
                                    perf_mode=DR)
                        # drains land in fb-pair tiles; one DMA per pair
                        # halves the HWDGE instruction tax on the out path.
                        # The very last p-block instead uses single-tile
                        # DMAs and alternates drains onto the (by then
                        # idle) DVE so the end-of-kernel chain is short.
                        t = p * FB + fb
                        if fb % 2 == 0:
                            ot = outpool.tile([128, 2, chunk], BF16,
                                              tag="ot",
                                              name=f"ot_{j}_{p}_{fb}")
                        nc.scalar.activation(ot[:, fb % 2, :], ops[:], IDENT,
                                             bias=bias_sb[:, t:t + 1],
                                             scale=OS)
                        last_pair = (j == n_chunks - 1 and p == P - 1
                                     and fb >= FB - 2)
                        if last_pair:
                            # ship each half as its own DMA so the kernel's
                            # closing chain rides on a 128KB transfer
                            nc.sync.dma_start(
                                out[t * 128:(t + 1) * 128,
                                    j * chunk:(j + 1) * chunk],
                                ot[:, fb % 2, :])
                        elif fb % 2 == 1:
                            t0 = p * FB + fb - 1
                            nc.sync.dma_start(
                                out[t0 * 128:(t0 + 2) * 128,
                                    j * chunk:(j + 1) * chunk]
                                .rearrange("(i p) n -> p i n", i=2, p=128),
                                ot[:])
                    yield
                for p in range(P):
                    for pr in range(PR):
                        del z8[(j, p, pr)]
                    for rt in range(RT):
                        del z[(j, p, rt)]

            # software pipeline: mm1(j+1) q-blocks interleave with mm2(j)
            # p-blocks so DMA demand (mm1-heavy) and ACT drains (mm2-heavy)
            # spread evenly and the PE never waits on a whole phase. The
            # two-block lead gives each chunk's mix/split chain time to
            # produce z8 before mm2 wants it, and leaves mm2 p-blocks after
            # the last mm1 to hide the final chunk's chain.
            prev = None
            for j in range(n_chunks):
                k = 0
                for _ in emit_mm1(j):
                    if prev is not None and k >= 1:
                        next(prev, None)
                    k += 1
                if prev is not None:
                    for _ in prev:
                        pass
                prev = emit_mm2(j)
            for _ in prev:
                pass

    nc.compile()
    return nc


def _f8_split(a, scale):
    import ml_dtypes
    F8NP = ml_dtypes.float8_e4m3
    s = np.asarray(a, dtype=np.float32) * scale
    hi = s.astype(F8NP)
    lo = (s - hi.astype(np.float32)).astype(F8NP)
    return np.ascontiguousarray(hi), np.ascontiguousarray(lo)


def _prep_in_maps(x, B, C, D, bias):
    x2 = np.asarray(x, dtype=np.float32).reshape(N_TOK, IN_F)
    CH, CL = _f8_split(
        np.asarray(C, dtype=np.float32).transpose(0, 2, 1).reshape(IN_F, R),
        WS)
    BH, BL = _f8_split(
        np.asarray(B, dtype=np.float32).transpose(0, 2, 1).reshape(P * R, OB),
        WS)
    DRm = np.ascontiguousarray(
        np.asarray(D, dtype=np.float32).transpose(2, 0, 1)
        .reshape(R, P * Q) / XS)
    BIASR = np.ascontiguousarray(
        np.asarray(bias, dtype=np.float32).reshape(OUT_F // 128, 128).T)

    in_maps = []
    for c in range(N_CORES):
        XH, XL = _f8_split(x2[c * N_CORE:(c + 1) * N_CORE].T, XS)
        in_maps.append({
            "xh8": XH, "xl8": XL, "ch8": CH, "cl8": CL,
            "bh8": BH, "bl8": BL, "dr": DRm, "biasr": BIASR,
        })
    return in_maps


def _run(in_maps, trace=False):
    global _cached_nc
    if _cached_nc is None:
        _cached_nc = _build()
    import time
    for attempt in range(3):
        try:
            return run_bass_kernel_spmd(
                _cached_nc, in_maps, list(range(N_CORES)), trace=trace)
        except Exception:
            # transient device errors (e.g. NRT_EXEC_UNIT_UNRECOVERABLE
            # from a previously wedged core) usually clear on retry
            if attempt == 2:
                raise
            time.sleep(5.0 * (attempt + 1))


def kernel(x, B, C, D, bias):
    lead = np.asarray(x).shape[:-1]
    res = _run(_prep_in_maps(x, B, C, D, bias))
    # per-core outputs are feature-major bf16 [OUT_F, n_core]; transpose back
    outs = [np.ascontiguousarray(res.results[c]["out"].T.astype(np.float32))
            for c in range(N_CORES)]
    return np.concatenate(outs, axis=0).reshape(*lead, OUT_F)
